# revision 20
# baseline (speedup 1.0000x reference)
"""TRN2 Bass kernel for nn_MultiHeadAttention (GQA + RoPE + causal, dense transformer).

Sharding: tensor-parallel over kv-head groups (TP=4; each core owns 2 kv heads
plus their 8 grouped q heads) x data-parallel over batch (DP=2) -> 8 cores.
The host sums the 4 partial o-projection outputs per batch element (the TP
all-reduce) and transposes back.

Per-core structure (chunked S pipeline, 4 chunks of 512 positions):
  ch: hs chunk -> qkv projections (resident weights, single-bank PSUM
      accumulators) -> RoPE -> block-diagonal Zk / row-duplicated qRd / vext
  t:  causal attention for 512 q rows: full-array K=128 score matmuls
      (kpos-split block-diagonal layout keeps the PE array fully busy so the
      HAM clock stays un-throttled), one merged exp per head-pair step on a
      2-bank PSUM tile, PV with an appended ones-column for the softmax
      denominators; then per-t normalization (SBUF-resident, reciprocals
      broadcast via DMA) and the t's slice of the o-projection.

Numerics: bf16 matmul paths with fp32 PSUM accumulation; softmax denominators
and normalization in f32/f32r.
"""
from contextlib import ExitStack

import numpy as np
import ml_dtypes

import concourse.bass as bass
import concourse.mybir as mybir
import concourse.tile as tile
from concourse import bacc
from concourse.bass_utils import run_bass_kernel_spmd
from concourse.masks import make_identity

F32 = mybir.dt.float32
F32R = mybir.dt.float32r
BF16 = mybir.dt.bfloat16
DT = BF16
AF = mybir.ActivationFunctionType

N_CORES = 8
B, S, D = 2, 2048, 2048
HQ_TOT, HKV_TOT, HD = 32, 8, 64
ROPE_BASE = 10000.0
TP = N_CORES // B          # 4 cores per batch element
HQ = HQ_TOT // TP          # 8 q heads per core
HKV = HKV_TOT // TP        # 2 kv heads per core


class Cfg:
    def __init__(self, S=2048, D=2048, HQ=8, HKV=2, HD=64):
        self.S, self.D, self.HQ, self.HKV, self.HD = S, D, HQ, HKV, HD
        self.QC = HQ * HD                  # q proj cols (512)
        self.KC = 2 * HKV * HD             # k+v proj cols (256)
        self.DK = D // 128                 # contraction k-tiles (16)
        self.T = S // 512                  # t-blocks of 512 q rows (4)
        self.KT = S // 128                 # kpos tiles of 128 (16)
        self.CH = 512                      # position-chunk width
        self.NCH = S // self.CH            # chunks (4)
        self.QM = self.QC // 128           # q proj m-tiles (4)
        self.NREP = HQ // HKV              # q heads per kv head (4)
        assert HD == 64 and self.QC % 128 == 0


def build(nc, cfg):
    c = cfg
    S, D = c.S, c.D

    hsT = nc.dram_tensor("hsT", [D, S], DT, kind="ExternalInput")
    wq = nc.dram_tensor("wq", [D, c.QC], DT, kind="ExternalInput")
    wkv = nc.dram_tensor("wkv", [D, c.KC], DT, kind="ExternalInput")
    wo = nc.dram_tensor("wo", [c.QC, D], DT, kind="ExternalInput")
    cos4_d = nc.dram_tensor("cos4", [128, S], DT, kind="ExternalInput")
    sinpm_d = nc.dram_tensor("sinpm", [128, S], DT, kind="ExternalInput")
    logmask_d = nc.dram_tensor("logmask", [128, c.KT], F32, kind="ExternalInput")
    onescol_d = nc.dram_tensor("onescol", [128, 1], F32R, kind="ExternalInput")
    outT = nc.dram_tensor("outT", [D, S], F32, kind="ExternalOutput")

    ctx = ExitStack()
    with tile.TileContext(nc) as tc:
        consts = ctx.enter_context(tc.tile_pool(name="consts", bufs=1))
        big = ctx.enter_context(tc.tile_pool(name="big", bufs=1))
        hspool = ctx.enter_context(tc.tile_pool(name="hspool", bufs=6))
        kcpool = ctx.enter_context(tc.tile_pool(name="kcpool", bufs=2))
        qcpool = ctx.enter_context(tc.tile_pool(name="qcpool", bufs=4))
        rppool = ctx.enter_context(tc.tile_pool(name="rppool", bufs=4))
        rpool = ctx.enter_context(tc.tile_pool(name="rpool", bufs=2))
        prpool = ctx.enter_context(tc.tile_pool(name="prpool", bufs=5))
        apool = ctx.enter_context(tc.tile_pool(name="apool", bufs=8))
        bbpool = ctx.enter_context(tc.tile_pool(name="bbpool", bufs=2))
        opool = ctx.enter_context(tc.tile_pool(name="opool", bufs=3))
        ps2pool = ctx.enter_context(tc.tile_pool(name="ps2", bufs=2, space="PSUM"))
        papool = ctx.enter_context(tc.tile_pool(name="papl", bufs=2, space="PSUM"))
        pjpool = ctx.enter_context(tc.tile_pool(name="pjpl", bufs=2, space="PSUM"))

        # ---- weights + first hs chunk first (minimize PE lead-in) ----
        wkv_big = consts.tile([128, c.DK * c.KC], DT, name="wkvbig")
        nc.sync.dma_start(
            wkv_big[:].rearrange("p (kk q) -> p kk q", kk=c.DK),
            wkv[:, :].rearrange("(kk p) q -> p kk q", p=128))

        def load_hs(ch):
            col0 = ch * c.CH
            groups = []
            for g in range(4):
                hg = hspool.tile([128, 4 * 512], DT, tag="hs",
                                 name=f"hs{ch}_{g}")
                nc.sync.dma_start(
                    hg[:].rearrange("p (kk q) -> p kk q", kk=4),
                    hsT[g * 512:(g + 1) * 512, col0:col0 + 512]
                    .rearrange("(kk p) q -> p kk q", p=128))
                groups.append(hg)
            return [groups[k // 4][:, (k % 4) * 512:(k % 4 + 1) * 512]
                    for k in range(c.DK)]

        hs0 = load_hs(0)
        cos4 = consts.tile([128, S], DT)
        nc.sync.dma_start(cos4[:], cos4_d[:])
        sinpm = consts.tile([128, S], DT)
        nc.sync.dma_start(sinpm[:], sinpm_d[:])
        logmask = consts.tile([128, c.KT], F32)
        nc.sync.dma_start(logmask[:], logmask_d[:])
        onescol = consts.tile([128, 1], F32R)
        nc.sync.dma_start(onescol[:], onescol_d[:])
        ident = consts.tile([128, 128], DT)
        make_identity(nc, ident[:])
        tri01 = consts.tile([128, 128], DT)
        nc.vector.memset(tri01[:], 1.0)
        nc.gpsimd.affine_select(
            out=tri01[:], in_=tri01[:], compare_op=mybir.AluOpType.is_ge,
            fill=0.0, base=0, channel_multiplier=-1, pattern=[[1, 128]])
        wq_grp = []
        for g in range(4):
            wg = consts.tile([128, 4 * c.QC], DT, name=f"wqgrp{g}")
            nc.sync.dma_start(
                wg[:].rearrange("p (kk q) -> p kk q", kk=4),
                wq[g * 512:(g + 1) * 512, :].rearrange("(kk p) q -> p kk q",
                                                       p=128))
            wq_grp.append(wg)
        wq_res = [wq_grp[k // 4][:, (k % 4) * c.QC:(k % 4 + 1) * c.QC]
                  for k in range(c.DK)]
        wo_res = []

        def load_wo():
            for k in range(c.QC // 128):
                wr = consts.tile([128, D], DT, name=f"wores{k}")
                nc.sync.dma_start(wr[:], wo[k * 128:(k + 1) * 128, :])
                wo_res.append(wr)

        # ---- resident tensors ----
        # Zk[j]: block-diagonal K layout. Column p of kpos-tile u holds
        # k-position u*128+p; its 64 head-dims sit at rows 0-63 when p < 64
        # and rows 64-127 when p >= 64 (zeros elsewhere), so score matmuls
        # against the row-duplicated qRd contract over all 128 partitions.
        Zk = [big.tile([128, S], DT, name=f"Zk{j}") for j in range(c.HKV)]
        qRd = [big.tile([128, S], DT, name=f"qRd{h}") for h in range(c.HQ)]
        vext = [big.tile([128, 2 * 65], DT, name=f"vext{u}") for u in range(c.KT)]
        attnT = [big.tile([128, S], DT, name=f"attnT{k}") for k in range(c.QC // 128)]
        denoms = big.tile([128, 512], F32, name="denoms")
        recips = big.tile([128, 512], F32, name="recips")
        for j in range(c.HKV):
            nc.gpsimd.memset(Zk[j][:], 0.0)

        wkv_res = [wkv_big[:, k * c.KC:(k + 1) * c.KC] for k in range(c.DK)]

        # ---- per-chunk projection + rope ----
        def rope(dst, src, col0):
            sl = slice(col0, col0 + 512)
            P = rpool.tile([128, 512], DT, tag="P")
            for blk in range(4):
                psrc = (blk ^ 1) * 32  # partner 32-block
                nc.scalar.dma_start(P[blk * 32:blk * 32 + 32, :],
                                    src[psrc:psrc + 32, :])
            m1 = rpool.tile([128, 512], DT, tag="m1")
            nc.vector.tensor_mul(m1[:], src[:], cos4[:, sl])
            m2 = rpool.tile([128, 512], DT, tag="m2")
            nc.vector.tensor_mul(m2[:], P[:], sinpm[:, sl])
            nc.vector.tensor_add(dst[:], m1[:], m2[:])

        def proj_col(w_res, msl, hs, name):
            pj = pjpool.tile([128, 512], F32, name=name, tag="pj")
            for k in range(c.DK):
                nc.tensor.matmul(pj[:], w_res[k][:, msl], hs[k],
                                 start=(k == 0), stop=(k == c.DK - 1))
            return pj

        def chunk(ch, hs):
            col0 = ch * c.CH
            # k heads -> rope -> Zk diagonal blocks
            pj = proj_col(wkv_res, slice(0, 128), hs, f"pk{ch}")
            kc = kcpool.tile([128, 512], DT, tag="kc", name=f"kc{ch}")
            nc.vector.tensor_copy(kc[:], pj[:])
            kr = kcpool.tile([128, 512], DT, tag="kr", name=f"kr{ch}")
            rope(kr, kc, col0)
            for j in range(c.HKV):
                for half in range(2):
                    nc.vector.tensor_copy(
                        Zk[j][half * 64:half * 64 + 64, col0:col0 + 512]
                        .rearrange("p (u two ccc) -> p u two ccc",
                                   two=2, ccc=64)[:, :, half, :],
                        kr[j * 64:(j + 1) * 64, :]
                        .rearrange("p (u two ccc) -> p u two ccc",
                                   two=2, ccc=64)[:, :, half, :])
            # q heads -> rope -> duplicated qRd
            for m in range(c.QM):
                pj = proj_col(wq_res, slice(m * 128, (m + 1) * 128), hs,
                              f"pq{ch}_{m}")
                qc = qcpool.tile([128, 512], DT, tag="qc", name=f"qc{ch}_{m}")
                nc.vector.tensor_copy(qc[:], pj[:])
                qr = rppool.tile([128, 512], DT, tag="qr", name=f"qr{ch}_{m}")
                rope(qr, qc, col0)
                for hh in range(2):
                    src = qr[hh * 64:hh * 64 + 64, :]
                    for half in range(2):
                        nc.scalar.dma_start(
                            qRd[2 * m + hh][half * 64:half * 64 + 64,
                                            col0:col0 + 512], src)
            # v heads -> transpose -> vext (with ones column)
            pj = proj_col(wkv_res, slice(128, 256), hs, f"pv{ch}")
            vc = kcpool.tile([128, 512], DT, tag="vc", name=f"vc{ch}")
            nc.vector.tensor_copy(vc[:], pj[:])
            for uu in range(4):
                u = 4 * ch + uu
                ps_t = kcpool.tile([128, 128], DT, name=f"pt{u}", tag="vt")
                nc.sync.dma_start(ps_t[:], vc[:, uu * 128:(uu + 1) * 128],
                                  transpose=True)
                dst = vext[u][:].rearrange("p (j cc) -> p j cc", j=2)[:, :, 0:64]
                vsrc = ps_t[:].rearrange("p (j cc) -> p j cc", j=2)
                nc.vector.tensor_copy(dst, vsrc)
                nc.vector.tensor_copy(
                    vext[u][:].rearrange("p (j cc) -> p j cc", j=2)[:, :, 64:65],
                    onescol[:].rearrange("p (j cc) -> p j cc", j=1)
                    .broadcast_to((128, 2, 1)))

        # ---- attention for one t-block of 512 q rows ----
        scale = float(c.HD) ** -0.5

        def attention_t(t):
            for hp in range(c.HQ // 2):
                j = hp // 2
                nu = 4 * t + 4
                pa = [papool.tile([65, 512], F32, name=f"pa{2 * hp + hh}_{t}",
                                  tag="pa") for hh in range(2)]
                for u in range(nu):
                    w = u - 4 * t
                    ncols = 512 if w < 0 else 512 - 128 * w
                    c0 = 512 - ncols
                    s2 = ps2pool.tile([128, 1024], F32, name=f"s2_{hp}_{t}_{u}",
                                      tag="s2")
                    for hh in range(2):
                        nc.tensor.matmul(
                            s2[:, hh * 512 + c0:(hh + 1) * 512],
                            Zk[j][:, u * 128:(u + 1) * 128],
                            qRd[2 * hp + hh][:, t * 512 + c0:(t + 1) * 512],
                            start=True, stop=True)
                    pr2 = prpool.tile([128, 1024], DT, tag="pr")
                    if c0 == 0:
                        nc.scalar.activation(pr2[:], s2[:], AF.Exp,
                                             bias=logmask[:, u:u + 1],
                                             scale=scale)
                    else:
                        view = lambda x: x[:].rearrange(
                            "p (h q) -> p h q", h=2)[:, :, c0:512]
                        nc.scalar.activation(view(pr2), view(s2), AF.Exp,
                                             bias=logmask[:, u:u + 1],
                                             scale=scale)
                    if w >= 0:
                        for hh in range(2):
                            base = hh * 512 + c0
                            nc.vector.tensor_mul(pr2[:, base:base + 128],
                                                 pr2[:, base:base + 128],
                                                 tri01[:])
                    for hh in range(2):
                        nc.tensor.matmul(
                            pa[hh][:, c0:512],
                            vext[u][:, j * 65:(j + 1) * 65],
                            pr2[:, hh * 512 + c0:(hh + 1) * 512],
                            start=(u == 0), stop=(u == nu - 1))
                for hh in range(2):
                    h = 2 * hp + hh
                    a_sb = apool.tile([65, 512], F32, tag="asb",
                                      name=f"a{h}_{t}")
                    nc.vector.tensor_copy(a_sb[:], pa[hh][:])
                    drow = (t % 2) * 64 + (h // 4) * 32 + (h % 4)
                    nc.sync.dma_start(denoms[drow:drow + 1, :],
                                      a_sb[64:65, :])
                    a_tiles[(h, t)] = a_sb

        a_tiles = {}

        def normalize_t(t):
            for half in range(2):
                r0 = (t % 2) * 64 + half * 32
                with nc.allow_low_precision(reason="softmax denominators"):
                    nc.vector.reciprocal(recips[r0:r0 + 4, :],
                                         denoms[r0:r0 + 4, :])
                for hh in range(4):
                    h = half * 4 + hh
                    rr0 = bbpool.tile([1, 512], F32, tag="rr0")
                    nc.sync.dma_start(rr0[:], recips[r0 + hh:r0 + hh + 1, :])
                    bb = bbpool.tile([64, 512], F32, tag="bb")
                    nc.gpsimd.partition_broadcast(bb[:], rr0[:])
                    a_sb = a_tiles.pop((h, t))
                    nc.vector.tensor_mul(
                        attnT[h // 2][(h % 2) * 64:(h % 2) * 64 + 64,
                                      t * 512:(t + 1) * 512],
                        a_sb[0:64, :], bb[:])

        def oproj_t(t):
            KO = c.QC // 128
            for mD in range(D // 128):
                po = pjpool.tile([128, 512], F32, name=f"po{mD}_{t}", tag="pj")
                for k in range(KO):
                    nc.tensor.matmul(po[:],
                                     wo_res[k][:, mD * 128:(mD + 1) * 128],
                                     attnT[k][:, t * 512:(t + 1) * 512],
                                     start=(k == 0), stop=(k == KO - 1))
                osb = opool.tile([128, 512], F32, tag="osb")
                if t >= 2:
                    nc.scalar.copy(osb[:], po[:])
                else:
                    nc.vector.tensor_copy(osb[:], po[:])
                nc.sync.dma_start(outT[mD * 128:(mD + 1) * 128,
                                       t * 512:(t + 1) * 512], osb[:])

        # ---- schedule ----
        chunk(0, hs0)
        attention_t(0)
        chunk(1, load_hs(1))
        load_wo()
        attention_t(1)
        normalize_t(0)
        oproj_t(0)
        chunk(2, load_hs(2))
        attention_t(2)
        normalize_t(1)
        oproj_t(1)
        chunk(3, load_hs(3))
        attention_t(3)
        normalize_t(2)
        oproj_t(2)
        normalize_t(3)
        oproj_t(3)
        ctx.close()
    return nc


def _host_prep(hidden_states, attention_mask, Wq, Wk, Wv, Wo):
    bf16 = ml_dtypes.bfloat16
    hs = np.asarray(hidden_states, np.float32)
    am = np.asarray(attention_mask)
    Wq = np.asarray(Wq, np.float32)
    Wk = np.asarray(Wk, np.float32)
    Wv = np.asarray(Wv, np.float32)
    Wo = np.asarray(Wo, np.float32)

    inv = 1.0 / (ROPE_BASE ** (np.arange(0, HD, 2, dtype=np.float64) / HD))
    freqs = np.arange(S, dtype=np.float64)[:, None] * inv[None, :]
    cosT = np.cos(freqs).T.astype(np.float32)
    sinT = np.sin(freqs).T.astype(np.float32)
    cos4 = np.tile(cosT, (4, 1))
    sign = np.repeat(np.array([-1.0, 1.0, -1.0, 1.0], np.float32), 32)[:, None]
    sinpm = np.tile(sinT, (4, 1)) * sign

    def perm_eo(wcols):  # head dims -> [evens | odds]
        return np.concatenate([wcols[:, 0::2], wcols[:, 1::2]], axis=1)

    hsT_b = [np.ascontiguousarray(hs[b].T).astype(bf16) for b in range(B)]
    lm_b = []
    for b in range(B):
        lm = np.where(am[b] > 0, 0.0, -1e30).astype(np.float32)
        lm_b.append(np.ascontiguousarray(lm.reshape(S // 128, 128).T))

    in_maps = []
    for core in range(N_CORES):
        b, g = core // TP, core % TP
        heads = range(g * HQ, (g + 1) * HQ)
        kvs = range(g * HKV, (g + 1) * HKV)
        wq_c = np.concatenate([perm_eo(Wq[:, h * HD:(h + 1) * HD]) for h in heads], 1)
        wk_c = np.concatenate([perm_eo(Wk[:, j * HD:(j + 1) * HD]) for j in kvs], 1)
        wv_c = np.concatenate([Wv[:, j * HD:(j + 1) * HD] for j in kvs], 1)
        wkv_c = np.ascontiguousarray(np.concatenate([wk_c, wv_c], 1))
        wo_c = np.ascontiguousarray(
            np.concatenate([Wo[h * HD:(h + 1) * HD, :] for h in heads], 0))
        in_maps.append({
            "hsT": hsT_b[b],
            "wq": np.ascontiguousarray(wq_c).astype(bf16),
            "wkv": wkv_c.astype(bf16),
            "wo": wo_c.astype(bf16),
            "cos4": cos4.astype(bf16),
            "sinpm": sinpm.astype(bf16),
            "logmask": lm_b[b],
            "onescol": np.ones((128, 1), np.float32),
        })
    return in_maps


_NC_CACHE = {}


def _get_nc():
    if "nc" not in _NC_CACHE:
        nc = bacc.Bacc("TRN2", target_bir_lowering=False, num_devices=N_CORES)
        build(nc, Cfg(S=S, D=D, HQ=HQ, HKV=HKV, HD=HD))
        nc.compile()
        _NC_CACHE["nc"] = nc
    return _NC_CACHE["nc"]


def kernel(hidden_states, attention_mask, Wq, Wk, Wv, Wo):
    nc = _get_nc()
    in_maps = _host_prep(hidden_states, attention_mask, Wq, Wk, Wv, Wo)
    res = run_bass_kernel_spmd(nc, in_maps, list(range(N_CORES)))
    out = np.zeros((B, S, D), np.float32)
    for core, r in enumerate(res.results):
        out[core // TP] += r["outT"].T
    return out


# revision 22
# speedup vs baseline: 1.0478x; 1.0478x over previous
"""TRN2 Bass kernel for nn_MultiHeadAttention (GQA + RoPE + causal, dense transformer).

Sharding: tensor-parallel over kv-head groups (TP=4; each core owns 2 kv heads
plus their 8 grouped q heads) x data-parallel over batch (DP=2) -> 8 cores.
The host sums the 4 partial o-projection outputs per batch element (the TP
all-reduce) and transposes back.

Per-core structure (chunked S pipeline, 4 chunks of 512 positions):
  ch: hs chunk -> qkv projections (resident weights, single-bank PSUM
      accumulators) -> RoPE -> block-diagonal Zk / row-duplicated qRd / vext
  t:  causal attention for 512 q rows: full-array K=128 score matmuls
      (kpos-split block-diagonal layout keeps the PE array fully busy so the
      HAM clock stays un-throttled), one merged exp per head-pair step on a
      2-bank PSUM tile, PV with an appended ones-column for the softmax
      denominators; then per-t normalization (SBUF-resident, reciprocals
      broadcast via DMA) and the t's slice of the o-projection.

Numerics: bf16 matmul paths with fp32 PSUM accumulation; softmax denominators
and normalization in f32/f32r.
"""
from contextlib import ExitStack

import numpy as np
import ml_dtypes

import concourse.bass as bass
import concourse.mybir as mybir
import concourse.tile as tile
from concourse import bacc
from concourse.bass_utils import run_bass_kernel_spmd
from concourse.masks import make_identity

F32 = mybir.dt.float32
F32R = mybir.dt.float32r
BF16 = mybir.dt.bfloat16
DT = BF16
AF = mybir.ActivationFunctionType

N_CORES = 8
B, S, D = 2, 2048, 2048
HQ_TOT, HKV_TOT, HD = 32, 8, 64
ROPE_BASE = 10000.0
TP = N_CORES // B          # 4 cores per batch element
HQ = HQ_TOT // TP          # 8 q heads per core
HKV = HKV_TOT // TP        # 2 kv heads per core


class Cfg:
    def __init__(self, S=2048, D=2048, HQ=8, HKV=2, HD=64):
        self.S, self.D, self.HQ, self.HKV, self.HD = S, D, HQ, HKV, HD
        self.QC = HQ * HD                  # q proj cols (512)
        self.KC = 2 * HKV * HD             # k+v proj cols (256)
        self.DK = D // 128                 # contraction k-tiles (16)
        self.T = S // 512                  # t-blocks of 512 q rows (4)
        self.KT = S // 128                 # kpos tiles of 128 (16)
        self.CH = 512                      # position-chunk width
        self.NCH = S // self.CH            # chunks (4)
        self.QM = self.QC // 128           # q proj m-tiles (4)
        self.NREP = HQ // HKV              # q heads per kv head (4)
        assert HD == 64 and self.QC % 128 == 0


def build(nc, cfg):
    c = cfg
    S, D = c.S, c.D

    hsT = nc.dram_tensor("hsT", [D, S], DT, kind="ExternalInput")
    wq = nc.dram_tensor("wq", [D, c.QC], DT, kind="ExternalInput")
    wkv = nc.dram_tensor("wkv", [D, c.KC], DT, kind="ExternalInput")
    wo = nc.dram_tensor("wo", [c.QC, D], DT, kind="ExternalInput")
    cos4_d = nc.dram_tensor("cos4", [128, S], DT, kind="ExternalInput")
    sinpm_d = nc.dram_tensor("sinpm", [128, S], DT, kind="ExternalInput")
    logmask_d = nc.dram_tensor("logmask", [128, c.KT], F32, kind="ExternalInput")
    onescol_d = nc.dram_tensor("onescol", [128, 1], F32R, kind="ExternalInput")
    outT = nc.dram_tensor("outT", [D, S], F32, kind="ExternalOutput")

    ctx = ExitStack()
    with tile.TileContext(nc) as tc:
        consts = ctx.enter_context(tc.tile_pool(name="consts", bufs=1))
        big = ctx.enter_context(tc.tile_pool(name="big", bufs=1))
        hspool = ctx.enter_context(tc.tile_pool(name="hspool", bufs=6))
        kcpool = ctx.enter_context(tc.tile_pool(name="kcpool", bufs=2))
        qcpool = ctx.enter_context(tc.tile_pool(name="qcpool", bufs=4))
        rppool = ctx.enter_context(tc.tile_pool(name="rppool", bufs=4))
        rpool = ctx.enter_context(tc.tile_pool(name="rpool", bufs=2))
        prpool = ctx.enter_context(tc.tile_pool(name="prpool", bufs=5))
        apool = ctx.enter_context(tc.tile_pool(name="apool", bufs=8))
        bbpool = ctx.enter_context(tc.tile_pool(name="bbpool", bufs=2))
        opool = ctx.enter_context(tc.tile_pool(name="opool", bufs=3))
        ps2pool = ctx.enter_context(tc.tile_pool(name="ps2", bufs=2, space="PSUM"))
        papool = ctx.enter_context(tc.tile_pool(name="papl", bufs=2, space="PSUM"))
        pjpool = ctx.enter_context(tc.tile_pool(name="pjpl", bufs=2, space="PSUM"))

        # ---- weights + first hs chunk first (minimize PE lead-in) ----
        wkv_big = consts.tile([128, c.DK * c.KC], DT, name="wkvbig")
        nc.sync.dma_start(
            wkv_big[:].rearrange("p (kk q) -> p kk q", kk=c.DK),
            wkv[:, :].rearrange("(kk p) q -> p kk q", p=128))

        def load_hs(ch):
            col0 = ch * c.CH
            groups = []
            for g in range(4):
                hg = hspool.tile([128, 4 * 512], DT, tag="hs",
                                 name=f"hs{ch}_{g}")
                nc.sync.dma_start(
                    hg[:].rearrange("p (kk q) -> p kk q", kk=4),
                    hsT[g * 512:(g + 1) * 512, col0:col0 + 512]
                    .rearrange("(kk p) q -> p kk q", p=128))
                groups.append(hg)
            return [groups[k // 4][:, (k % 4) * 512:(k % 4 + 1) * 512]
                    for k in range(c.DK)]

        hs0 = load_hs(0)
        cos4 = consts.tile([128, S], DT)
        nc.sync.dma_start(cos4[:], cos4_d[:])
        sinpm = consts.tile([128, S], DT)
        nc.sync.dma_start(sinpm[:], sinpm_d[:])
        logmask = consts.tile([128, c.KT], F32)
        nc.sync.dma_start(logmask[:], logmask_d[:])
        onescol = consts.tile([128, 1], F32R)
        nc.sync.dma_start(onescol[:], onescol_d[:])
        ident = consts.tile([128, 128], DT)
        make_identity(nc, ident[:])
        tri01 = consts.tile([128, 128], DT)
        nc.vector.memset(tri01[:], 1.0)
        nc.gpsimd.affine_select(
            out=tri01[:], in_=tri01[:], compare_op=mybir.AluOpType.is_ge,
            fill=0.0, base=0, channel_multiplier=-1, pattern=[[1, 128]])
        wq_grp = []
        for g in range(4):
            wg = consts.tile([128, 4 * c.QC], DT, name=f"wqgrp{g}")
            nc.sync.dma_start(
                wg[:].rearrange("p (kk q) -> p kk q", kk=4),
                wq[g * 512:(g + 1) * 512, :].rearrange("(kk p) q -> p kk q",
                                                       p=128))
            wq_grp.append(wg)
        wq_res = [wq_grp[k // 4][:, (k % 4) * c.QC:(k % 4 + 1) * c.QC]
                  for k in range(c.DK)]
        wo_res = []

        def load_wo():
            for k in range(c.QC // 128):
                wr = consts.tile([128, D], DT, name=f"wores{k}")
                nc.sync.dma_start(wr[:], wo[k * 128:(k + 1) * 128, :])
                wo_res.append(wr)

        # ---- resident tensors ----
        # Zk[j]: block-diagonal K layout. Column p of kpos-tile u holds
        # k-position u*128+p; its 64 head-dims sit at rows 0-63 when p < 64
        # and rows 64-127 when p >= 64 (zeros elsewhere), so score matmuls
        # against the row-duplicated qRd contract over all 128 partitions.
        Zk = [big.tile([128, S], DT, name=f"Zk{j}") for j in range(c.HKV)]
        qRd = [big.tile([128, S], DT, name=f"qRd{h}") for h in range(c.HQ)]
        vext = [big.tile([128, 2 * 65], DT, name=f"vext{u}") for u in range(c.KT)]
        attnT = [big.tile([128, S], DT, name=f"attnT{k}") for k in range(c.QC // 128)]
        denoms = big.tile([128, 512], F32, name="denoms")
        recips = big.tile([128, 512], F32, name="recips")
        for j in range(c.HKV):
            nc.gpsimd.memset(Zk[j][:], 0.0)

        wkv_res = [wkv_big[:, k * c.KC:(k + 1) * c.KC] for k in range(c.DK)]

        # ---- per-chunk projection + rope ----
        def rope(dst, src, col0):
            sl = slice(col0, col0 + 512)
            P = rpool.tile([128, 512], DT, tag="P")
            for blk in range(4):
                psrc = (blk ^ 1) * 32  # partner 32-block
                nc.scalar.dma_start(P[blk * 32:blk * 32 + 32, :],
                                    src[psrc:psrc + 32, :])
            m1 = rpool.tile([128, 512], DT, tag="m1")
            nc.vector.tensor_mul(m1[:], src[:], cos4[:, sl])
            m2 = rpool.tile([128, 512], DT, tag="m2")
            nc.vector.tensor_mul(m2[:], P[:], sinpm[:, sl])
            nc.vector.tensor_add(dst[:], m1[:], m2[:])

        def proj_col(w_res, msl, hs, name):
            pj = pjpool.tile([128, 512], F32, name=name, tag="pj")
            for k in range(c.DK):
                nc.tensor.matmul(pj[:], w_res[k][:, msl], hs[k],
                                 start=(k == 0), stop=(k == c.DK - 1))
            return pj

        def chunk(ch, hs):
            col0 = ch * c.CH
            # k heads -> rope -> Zk diagonal blocks
            pj = proj_col(wkv_res, slice(0, 128), hs, f"pk{ch}")
            kc = kcpool.tile([128, 512], DT, tag="kc", name=f"kc{ch}")
            nc.vector.tensor_copy(kc[:], pj[:])
            kr = kcpool.tile([128, 512], DT, tag="kr", name=f"kr{ch}")
            rope(kr, kc, col0)
            for j in range(c.HKV):
                for half in range(2):
                    nc.vector.tensor_copy(
                        Zk[j][half * 64:half * 64 + 64, col0:col0 + 512]
                        .rearrange("p (u two ccc) -> p u two ccc",
                                   two=2, ccc=64)[:, :, half, :],
                        kr[j * 64:(j + 1) * 64, :]
                        .rearrange("p (u two ccc) -> p u two ccc",
                                   two=2, ccc=64)[:, :, half, :])
            # q heads -> rope -> duplicated qRd (m=0 first: unblocks hp=0)
            def q_path(m):
                pj = proj_col(wq_res, slice(m * 128, (m + 1) * 128), hs,
                              f"pq{ch}_{m}")
                qc = qcpool.tile([128, 512], DT, tag="qc", name=f"qc{ch}_{m}")
                nc.vector.tensor_copy(qc[:], pj[:])
                qr = rppool.tile([128, 512], DT, tag="qr", name=f"qr{ch}_{m}")
                rope(qr, qc, col0)
                for hh in range(2):
                    src = qr[hh * 64:hh * 64 + 64, :]
                    for half in range(2):
                        nc.scalar.dma_start(
                            qRd[2 * m + hh][half * 64:half * 64 + 64,
                                            col0:col0 + 512], src)
            q_path(0)
            # v heads -> transpose -> vext (with ones column)
            pj = proj_col(wkv_res, slice(128, 256), hs, f"pv{ch}")
            vc = kcpool.tile([128, 512], DT, tag="vc", name=f"vc{ch}")
            nc.vector.tensor_copy(vc[:], pj[:])
            for uu in range(4):
                u = 4 * ch + uu
                ps_t = ps2pool.tile([128, 128], DT, name=f"pt{u}", tag="s2")
                nc.tensor.transpose(ps_t[:], vc[:, uu * 128:(uu + 1) * 128],
                                    ident[:])
                dst = vext[u][:].rearrange("p (j cc) -> p j cc", j=2)[:, :, 0:64]
                vsrc = ps_t[:].rearrange("p (j cc) -> p j cc", j=2)
                nc.vector.tensor_copy(dst, vsrc)
                nc.vector.tensor_copy(
                    vext[u][:].rearrange("p (j cc) -> p j cc", j=2)[:, :, 64:65],
                    onescol[:].rearrange("p (j cc) -> p j cc", j=1)
                    .broadcast_to((128, 2, 1)))
            for m in range(1, c.QM):
                q_path(m)

        # ---- attention for one t-block of 512 q rows ----
        scale = float(c.HD) ** -0.5

        def attention_t(t):
            for hp in range(c.HQ // 2):
                j = hp // 2
                nu = 4 * t + 4
                pa = [papool.tile([65, 512], F32, name=f"pa{2 * hp + hh}_{t}",
                                  tag="pa") for hh in range(2)]
                for u in range(nu):
                    w = u - 4 * t
                    ncols = 512 if w < 0 else 512 - 128 * w
                    c0 = 512 - ncols
                    s2 = ps2pool.tile([128, 1024], F32, name=f"s2_{hp}_{t}_{u}",
                                      tag="s2")
                    for hh in range(2):
                        nc.tensor.matmul(
                            s2[:, hh * 512 + c0:(hh + 1) * 512],
                            Zk[j][:, u * 128:(u + 1) * 128],
                            qRd[2 * hp + hh][:, t * 512 + c0:(t + 1) * 512],
                            start=True, stop=True)
                    pr2 = prpool.tile([128, 1024], DT, tag="pr")
                    if c0 == 0:
                        nc.scalar.activation(pr2[:], s2[:], AF.Exp,
                                             bias=logmask[:, u:u + 1],
                                             scale=scale)
                    else:
                        view = lambda x: x[:].rearrange(
                            "p (h q) -> p h q", h=2)[:, :, c0:512]
                        nc.scalar.activation(view(pr2), view(s2), AF.Exp,
                                             bias=logmask[:, u:u + 1],
                                             scale=scale)
                    if w >= 0:
                        for hh in range(2):
                            base = hh * 512 + c0
                            nc.vector.tensor_mul(pr2[:, base:base + 128],
                                                 pr2[:, base:base + 128],
                                                 tri01[:])
                    for hh in range(2):
                        nc.tensor.matmul(
                            pa[hh][:, c0:512],
                            vext[u][:, j * 65:(j + 1) * 65],
                            pr2[:, hh * 512 + c0:(hh + 1) * 512],
                            start=(u == 0), stop=(u == nu - 1))
                for hh in range(2):
                    h = 2 * hp + hh
                    a_sb = apool.tile([65, 512], F32, tag="asb",
                                      name=f"a{h}_{t}")
                    nc.vector.tensor_copy(a_sb[:], pa[hh][:])
                    drow = (t % 2) * 64 + (h // 4) * 32 + (h % 4)
                    nc.sync.dma_start(denoms[drow:drow + 1, :],
                                      a_sb[64:65, :])
                    a_tiles[(h, t)] = a_sb

        a_tiles = {}

        def normalize_t(t):
            for half in range(2):
                r0 = (t % 2) * 64 + half * 32
                with nc.allow_low_precision(reason="softmax denominators"):
                    nc.vector.reciprocal(recips[r0:r0 + 4, :],
                                         denoms[r0:r0 + 4, :])
                for hh in range(4):
                    h = half * 4 + hh
                    rr0 = bbpool.tile([1, 512], F32, tag="rr0")
                    nc.sync.dma_start(rr0[:], recips[r0 + hh:r0 + hh + 1, :])
                    bb = bbpool.tile([64, 512], F32, tag="bb")
                    nc.gpsimd.partition_broadcast(bb[:], rr0[:])
                    a_sb = a_tiles.pop((h, t))
                    nc.vector.tensor_mul(
                        attnT[h // 2][(h % 2) * 64:(h % 2) * 64 + 64,
                                      t * 512:(t + 1) * 512],
                        a_sb[0:64, :], bb[:])

        def oproj_t(t):
            KO = c.QC // 128
            for mD in range(D // 128):
                po = pjpool.tile([128, 512], F32, name=f"po{mD}_{t}", tag="pj")
                for k in range(KO):
                    nc.tensor.matmul(po[:],
                                     wo_res[k][:, mD * 128:(mD + 1) * 128],
                                     attnT[k][:, t * 512:(t + 1) * 512],
                                     start=(k == 0), stop=(k == KO - 1))
                osb = opool.tile([128, 512], F32, tag="osb")
                if t >= 2:
                    nc.scalar.copy(osb[:], po[:])
                else:
                    nc.vector.tensor_copy(osb[:], po[:])
                nc.sync.dma_start(outT[mD * 128:(mD + 1) * 128,
                                       t * 512:(t + 1) * 512], osb[:])

        # ---- schedule ----
        chunk(0, hs0)
        attention_t(0)
        chunk(1, load_hs(1))
        load_wo()
        attention_t(1)
        normalize_t(0)
        oproj_t(0)
        chunk(2, load_hs(2))
        attention_t(2)
        normalize_t(1)
        oproj_t(1)
        chunk(3, load_hs(3))
        attention_t(3)
        normalize_t(2)
        oproj_t(2)
        normalize_t(3)
        oproj_t(3)
        ctx.close()
    return nc


def _host_prep(hidden_states, attention_mask, Wq, Wk, Wv, Wo):
    bf16 = ml_dtypes.bfloat16
    hs = np.asarray(hidden_states, np.float32)
    am = np.asarray(attention_mask)
    Wq = np.asarray(Wq, np.float32)
    Wk = np.asarray(Wk, np.float32)
    Wv = np.asarray(Wv, np.float32)
    Wo = np.asarray(Wo, np.float32)

    inv = 1.0 / (ROPE_BASE ** (np.arange(0, HD, 2, dtype=np.float64) / HD))
    freqs = np.arange(S, dtype=np.float64)[:, None] * inv[None, :]
    cosT = np.cos(freqs).T.astype(np.float32)
    sinT = np.sin(freqs).T.astype(np.float32)
    cos4 = np.tile(cosT, (4, 1))
    sign = np.repeat(np.array([-1.0, 1.0, -1.0, 1.0], np.float32), 32)[:, None]
    sinpm = np.tile(sinT, (4, 1)) * sign

    def perm_eo(wcols):  # head dims -> [evens | odds]
        return np.concatenate([wcols[:, 0::2], wcols[:, 1::2]], axis=1)

    hsT_b = [np.ascontiguousarray(hs[b].T).astype(bf16) for b in range(B)]
    lm_b = []
    for b in range(B):
        lm = np.where(am[b] > 0, 0.0, -1e30).astype(np.float32)
        lm_b.append(np.ascontiguousarray(lm.reshape(S // 128, 128).T))

    in_maps = []
    for core in range(N_CORES):
        b, g = core // TP, core % TP
        heads = range(g * HQ, (g + 1) * HQ)
        kvs = range(g * HKV, (g + 1) * HKV)
        wq_c = np.concatenate([perm_eo(Wq[:, h * HD:(h + 1) * HD]) for h in heads], 1)
        wk_c = np.concatenate([perm_eo(Wk[:, j * HD:(j + 1) * HD]) for j in kvs], 1)
        wv_c = np.concatenate([Wv[:, j * HD:(j + 1) * HD] for j in kvs], 1)
        wkv_c = np.ascontiguousarray(np.concatenate([wk_c, wv_c], 1))
        wo_c = np.ascontiguousarray(
            np.concatenate([Wo[h * HD:(h + 1) * HD, :] for h in heads], 0))
        in_maps.append({
            "hsT": hsT_b[b],
            "wq": np.ascontiguousarray(wq_c).astype(bf16),
            "wkv": wkv_c.astype(bf16),
            "wo": wo_c.astype(bf16),
            "cos4": cos4.astype(bf16),
            "sinpm": sinpm.astype(bf16),
            "logmask": lm_b[b],
            "onescol": np.ones((128, 1), np.float32),
        })
    return in_maps


_NC_CACHE = {}


def _get_nc():
    if "nc" not in _NC_CACHE:
        nc = bacc.Bacc("TRN2", target_bir_lowering=False, num_devices=N_CORES)
        build(nc, Cfg(S=S, D=D, HQ=HQ, HKV=HKV, HD=HD))
        nc.compile()
        _NC_CACHE["nc"] = nc
    return _NC_CACHE["nc"]


def kernel(hidden_states, attention_mask, Wq, Wk, Wv, Wo):
    nc = _get_nc()
    in_maps = _host_prep(hidden_states, attention_mask, Wq, Wk, Wv, Wo)
    res = run_bass_kernel_spmd(nc, in_maps, list(range(N_CORES)))
    out = np.zeros((B, S, D), np.float32)
    for core, r in enumerate(res.results):
        out[core // TP] += r["outT"].T
    return out
